# revision 12
# baseline (speedup 1.0000x reference)
"""Trainium2 Bass kernel for nn_DecoderBlock (dense transformer decoder block).

Sharding: data-parallel over batch (8 batch elements -> 8 NeuronCores), no
collectives. Each core computes one full decoder block on [S=1024, D=1024].

Per-core strategy (bf16 datapath):
  - activations kept feature-major ([D, S]) in bf16; every Linear is
    out = W_tile.T @ actT with W streamed from HBM in bf16 (cast on host)
  - weights loaded full-tensor with >=2KB contiguous lines (one DMA per
    [D,1024] column chunk) instead of 512B column strips
  - LN1 in natural layout (free-dim bn_stats, fp32) fused into the input
    transpose; LN2/LN3 feature-major with partition-dim stats via
    ones-matmuls (fp32 squares) and K=1 broadcast matmuls
  - softmax without max-subtraction (scores are small); causal mask via
    precomputed 0/1 bf16 mask tiles multiplied on the exp'd tiles (DVE);
    V carries an appended ones column so softmax denominators fall out of
    the attn@v matmul
  - matmuls bf16 x bf16 -> fp32 PSUM (1 cycle/row, same as fp32r, half the
    SBUF/HBM traffic); per-tensor quantization ~2e-3 rel, well inside the
    2e-2 gate
"""
import sys

sys.path.insert(0, '/opt/trn_rl_repo')

import contextlib

import ml_dtypes
import numpy as np

import concourse.bacc as bacc
import concourse.mybir as mybir
import concourse.tile as tile
from concourse.bass_utils import run_bass_kernel_spmd
from concourse.masks import make_identity

f32 = mybir.dt.float32
f32r = mybir.dt.float32r
bf16 = mybir.dt.bfloat16
AF = mybir.ActivationFunctionType
ALU = mybir.AluOpType

B, S, D, H, HD, FF = 8, 1024, 1024, 16, 64, 4096
ST = S // 128   # 8
DT = D // 128   # 8
FT = FF // 128  # 32
EPS = 1e-5
ISQ = 1.0 / 8.0  # 1/sqrt(HD)

W_NAMES = ['sa_wq', 'sa_wk', 'sa_wv', 'sa_wo', 'ca_wq', 'ca_wk', 'ca_wv', 'ca_wo']
B_NAMES = ['sa_bq', 'sa_bk', 'sa_bv', 'sa_bo', 'ca_bq', 'ca_bk', 'ca_bv', 'ca_bo']
LN_NAMES = ['ln1_g', 'ln1_b', 'ln2_g', 'ln2_b', 'ln3_g', 'ln3_b']


def _build(iters=1):
    nc = bacc.Bacc("TRN2", target_bir_lowering=False, debug=False, num_devices=8)

    dec_d = nc.dram_tensor("decoder", [S, D], f32, kind="ExternalInput").ap()
    enc_d = nc.dram_tensor("encoder", [S, D], f32, kind="ExternalInput").ap()
    wd = {n: nc.dram_tensor(n, [D, D], bf16, kind="ExternalInput").ap() for n in W_NAMES}
    bd = {n: nc.dram_tensor(n, [1, D] if n.endswith('bv') else [D], f32,
                            kind="ExternalInput").ap() for n in B_NAMES}
    lnd = {n: nc.dram_tensor(n, [D], f32, kind="ExternalInput").ap() for n in LN_NAMES}
    w1_d = nc.dram_tensor("ffn_w1", [D, FF], bf16, kind="ExternalInput").ap()
    b1_d = nc.dram_tensor("ffn_b1", [FF], f32, kind="ExternalInput").ap()
    w2_d = nc.dram_tensor("ffn_w2", [FF, D], bf16, kind="ExternalInput").ap()
    b2_d = nc.dram_tensor("ffn_b2", [D], f32, kind="ExternalInput").ap()
    out_d = nc.dram_tensor("out", [S, D], f32, kind="ExternalOutput").ap()

    with tile.TileContext(nc) as tc, \
            nc.allow_low_precision(reason="bf16 matmul pipeline by design"):
        _body(nc, tc, dec_d, enc_d, wd, bd, lnd, w1_d, b1_d, w2_d, b2_d, out_d, iters)
    nc.compile()
    return nc


def _body(nc, tc, dec_d, enc_d, wd, bd, lnd, w1_d, b1_d, w2_d, b2_d, out_d, iters):
    ctx = contextlib.ExitStack()
    with ctx:
        persist = ctx.enter_context(tc.tile_pool(name="persist", bufs=1))
        big = ctx.enter_context(tc.tile_pool(name="big", bufs=1))
        wf = ctx.enter_context(tc.tile_pool(name="wf", bufs=3))
        stri = ctx.enter_context(tc.tile_pool(name="stri", bufs=2))
        att = ctx.enter_context(tc.tile_pool(name="att", bufs=3))
        sm = ctx.enter_context(tc.tile_pool(name="sm", bufs=2))
        ps_a = ctx.enter_context(tc.tile_pool(name="ps_a", bufs=4, space="PSUM"))
        ps_r = ctx.enter_context(tc.tile_pool(name="ps_r", bufs=4, space="PSUM"))

        # ---- persistent constants ----
        ones_f = persist.tile([128, 8], f32, tag="ones_f")
        nc.vector.memset(ones_f, 1.0)
        ones_col_b = persist.tile([128, 1], bf16, tag="ones_col_b")
        nc.vector.tensor_copy(ones_col_b, ones_f[:, 0:1])
        ones_col_r = persist.tile([128, 1], f32r, tag="ones_col_r")
        nc.vector.tensor_copy(ones_col_r, ones_f[:, 0:1])
        onesr_f = persist.tile([1, 128], f32, tag="onesr_f")
        nc.vector.memset(onesr_f, 1.0)
        ones_row = persist.tile([1, 128], f32r, tag="ones_row")
        nc.vector.tensor_copy(ones_row, onesr_f)
        ident = persist.tile([128, 128], f32, tag="ident")
        make_identity(nc, ident)
        ident_r = persist.tile([128, 128], f32r, tag="ident_r")
        nc.vector.tensor_copy(ident_r, ident)
        ident_b = persist.tile([128, 128], bf16, tag="ident_b")
        nc.vector.tensor_copy(ident_b, ident)
        eps1 = persist.tile([1, 1], f32, tag="eps1")
        nc.vector.memset(eps1, EPS)
        eps128 = persist.tile([128, 1], f32, tag="eps128")
        nc.vector.memset(eps128, EPS)

        # causal 0/1 masks: mask[i][p, f] = (f - 128*i - p >= 0)
        mask_f = sm.tile([128, 512], f32, tag="sq")
        masks = []
        for i in range(4):
            nc.vector.memset(mask_f, 1.0)
            nc.gpsimd.affine_select(
                out=mask_f, in_=mask_f, compare_op=ALU.is_ge, fill=0.0,
                base=-128 * i, pattern=[[1, 512]], channel_multiplier=-1)
            mb = persist.tile([128, 512], bf16, tag=f"mask{i}", name=f"mask{i}")
            nc.vector.tensor_copy(mb, mask_f)
            masks.append(mb)

        bias_t = {}
        for n in ['sa_bq', 'sa_bk', 'sa_bo', 'ca_bq', 'ca_bk', 'ca_bo']:
            bias_t[n] = persist.tile([128, DT], f32, tag=n, name=n)
            nc.sync.dma_start(bias_t[n], bd[n].rearrange("(t p) -> p t", p=128))
        for n in LN_NAMES:
            bias_t[n] = persist.tile([128, DT], f32, tag=n, name=n)
            nc.sync.dma_start(bias_t[n], lnd[n].rearrange("(t p) -> p t", p=128))
        bias_t['ffn_b1'] = persist.tile([128, FT], f32, tag="ffn_b1", name="ffn_b1")
        nc.sync.dma_start(bias_t['ffn_b1'], b1_d.rearrange("(t p) -> p t", p=128))
        bias_t['ffn_b2'] = persist.tile([128, DT], f32, tag="ffn_b2", name="ffn_b2")
        nc.sync.dma_start(bias_t['ffn_b2'], b2_d.rearrange("(t p) -> p t", p=128))

        # V-bias broadcast to all partitions, once per attention (init-time)
        bcv_sb = {}
        for pre in ('sa_', 'ca_'):
            brow = persist.tile([1, D], f32r, tag=f"{pre}bvrow", name=f"{pre}bvrow")
            nc.sync.dma_start(brow, bd[pre + 'bv'].bitcast(f32r))
            bct = persist.tile([128, D], bf16, tag=f"{pre}bcv", name=f"{pre}bcv")
            for c in range(2):
                cs = slice(c * 512, (c + 1) * 512)
                bc = ps_r.tile([128, 512], f32, tag="r")
                nc.tensor.matmul(bc, ones_row, brow[:, cs],
                                 start=True, stop=True, skip_group_check=True)
                nc.vector.tensor_copy(bct[:, cs], bc)
            bcv_sb[pre] = bct

        # stats scratch rows (single-buffered, reused per LN/softmax call)
        a_row = persist.tile([1, S], f32r, tag="a_row")
        c_row = persist.tile([1, S], f32r, tag="c_row")
        rowA = persist.tile([1, 512], f32, tag="rowA")
        rowB = persist.tile([1, 512], f32, tag="rowB")
        rowC = persist.tile([1, 512], f32, tag="rowC")
        rec = persist.tile([1, 512], f32r, tag="rec")

        # ---- big bf16 activation buffers [128, 8, 1024] (2 MB each) ----
        # A: xT -> encT -> h[8:16]
        # Bb: x2T/yT -> FFN accumulator (out)
        # C: SA qT -> y2T/zT   (kT for CA, overwritten by proj_T dst)
        # R: repT (SA+CA) -> h[0:8]
        # Hb: SA kT -> CA qT  ... careful: see attention() call sites
        A = big.tile([128, DT, S], bf16, tag="A")
        Bb = big.tile([128, DT, S], bf16, tag="Bb")
        C = big.tile([128, DT, S], bf16, tag="C")
        R = big.tile([128, DT, S], bf16, tag="R")
        Hb = big.tile([128, DT, S], bf16, tag="Hb")
        vg = big.tile([128, ST, H, 65], bf16, tag="vg")

        def mm(out_ap, lhsT_ap, rhs_ap, start, stop):
            nc.tensor.matmul(out_ap, lhsT_ap, rhs_ap, start=start, stop=stop,
                             skip_group_check=True)

        def load_w_full(w_dram, col0, ncols, tag="wf"):
            # [in, out] weight chunk -> SBUF [128, k-tile, ncols] bf16,
            # 2KB+ contiguous lines (full rows of the chunk)
            t = wf.tile([128, DT, 1024], bf16, tag=tag)
            nc.sync.dma_start(
                t[:, :, 0:ncols],
                w_dram[:, col0:col0 + ncols].rearrange("(k p) q -> p k q", p=128))
            return t

        def proj_T(wt, bias, src_T, dst_T, residual=None):
            # dst[:, m, :] = wt[:, :, m-block].T @ src (+bias) (+residual)
            for m in range(DT):
                pss = [ps_a.tile([128, 512], f32, tag="a", name=f"pj{_c}")
                       for _c in range(2)]
                for k in range(DT):
                    for c in range(2):
                        cs = slice(c * 512, (c + 1) * 512)
                        mm(pss[c], wt[:, k, m * 128:(m + 1) * 128],
                           src_T[:, k, cs], k == 0, k == DT - 1)
                for c in range(2):
                    cs = slice(c * 512, (c + 1) * 512)
                    if residual is None:
                        nc.vector.tensor_scalar(
                            dst_T[:, m, cs], pss[c], bias[:, m:m + 1], None,
                            ALU.add)
                    else:
                        nc.vector.scalar_tensor_tensor(
                            dst_T[:, m, cs], pss[c], bias[:, m:m + 1],
                            residual[:, m, cs], ALU.add, ALU.add)

        def ln_partition(T, g_ap, b_ap):
            # in-place layernorm over the feature (partition-tiled) dim of T
            sums = [ps_r.tile([1, 512], f32, tag="r", name=f"sums{_c}") for _c in range(2)]
            ssqs = [ps_r.tile([1, 512], f32, tag="r", name=f"ssqs{_c}") for _c in range(2)]
            for t in range(DT):
                for c in range(2):
                    cs = slice(c * 512, (c + 1) * 512)
                    sq = sm.tile([128, 512], f32, tag="sq")
                    sqr = sq.bitcast(f32r)
                    nc.vector.tensor_mul(sqr, T[:, t, cs], T[:, t, cs])
                    mm(sums[c], ones_col_b, T[:, t, cs], t == 0, t == DT - 1)
                    mm(ssqs[c], ones_col_r, sqr, t == 0, t == DT - 1)
            for c in range(2):
                cs = slice(c * 512, (c + 1) * 512)
                nc.vector.tensor_scalar(rowA, sums[c], 1.0 / D, None, ALU.mult)  # mu
                nc.vector.tensor_scalar(rowB, ssqs[c], 1.0 / D, None, ALU.mult)  # E[x^2]
                nc.vector.scalar_tensor_tensor(rowC, rowA, -1.0, rowA, ALU.mult,
                                               ALU.mult)                          # -mu^2
                nc.vector.tensor_add(rowB, rowB, rowC)                            # var
                nc.scalar.activation(rowC, rowB, AF.Sqrt, bias=eps1)              # std
                nc.vector.reciprocal(rowB, rowC)                                  # rstd
                nc.vector.tensor_copy(a_row[:, cs], rowB)
                nc.vector.scalar_tensor_tensor(c_row[:, cs], rowA, -1.0, rowB,
                                               ALU.mult, ALU.mult)                # -mu*rstd
            bcA = [ps_a.tile([128, 512], f32, tag="a", name=f"bcA{_c}")
                   for _c in range(2)]
            bcC = [ps_a.tile([128, 512], f32, tag="a", name=f"bcC{_c}")
                   for _c in range(2)]
            for c in range(2):
                cs = slice(c * 512, (c + 1) * 512)
                mm(bcA[c], ones_row, a_row[:, cs], True, True)
                mm(bcC[c], ones_row, c_row[:, cs], True, True)
            for t in range(DT):
                for c in range(2):
                    cs = slice(c * 512, (c + 1) * 512)
                    tmp = sm.tile([128, 512], bf16, tag="lntmp", name="lntmp")
                    nc.vector.tensor_scalar(tmp, bcC[c], g_ap[:, t:t + 1],
                                            b_ap[:, t:t + 1], ALU.mult, ALU.add)
                    nc.vector.tensor_mul(T[:, t, cs], T[:, t, cs], bcA[c])
                    nc.vector.scalar_tensor_tensor(
                        T[:, t, cs], T[:, t, cs], g_ap[:, t:t + 1],
                        tmp, ALU.mult, ALU.add)

        def attention(src_q_T, src_kv_T, pre, causal, dst_T, residual_T,
                      qT, kT, repT):
            wq_d, wk_d, wv_d, wo_d = (wd[pre + n] for n in ('wq', 'wk', 'wv', 'wo'))
            bq, bk, bo = (bias_t[pre + n] for n in ('bq', 'bk', 'bo'))

            # Q / K projections (full weight loaded once, 2KB lines)
            for wmat_d, bmat, dst, srcx in ((wq_d, bq, qT, src_q_T),
                                            (wk_d, bk, kT, src_kv_T)):
                wt = load_w_full(wmat_d, 0, 1024)
                for m in range(DT):
                    pss = [ps_a.tile([128, 512], f32, tag="a", name=f"qk{_c}")
                           for _c in range(2)]
                    for k in range(DT):
                        for c in range(2):
                            cs = slice(c * 512, (c + 1) * 512)
                            mm(pss[c], wt[:, k, m * 128:(m + 1) * 128],
                               srcx[:, k, cs], k == 0, k == DT - 1)
                    for c in range(2):
                        cs = slice(c * 512, (c + 1) * 512)
                        nc.vector.tensor_scalar(
                            dst[:, m, cs], pss[c], bmat[:, m:m + 1], None,
                            ALU.add)

            # V in natural layout (+bias broadcast, +ones column)
            wvt = load_w_full(wv_d, 0, 1024)
            bcv = bcv_sb[pre]
            for st in range(ST):
                for c in range(2):
                    cs = slice(c * 512, (c + 1) * 512)
                    psv = ps_r.tile([128, 512], f32, tag="r")
                    for k in range(DT):
                        mm(psv, src_kv_T[:, k, st * 128:(st + 1) * 128],
                           wvt[:, k, cs], k == 0, k == DT - 1)
                    nc.vector.tensor_add(
                        vg[:, st, c * 8:(c + 1) * 8, 0:64],
                        psv.rearrange("p (h e) -> p h e", h=8),
                        bcv[:, cs].rearrange("p (h e) -> p h e", h=8))
                    nc.vector.tensor_copy(
                        vg[:, st, c * 8:(c + 1) * 8, 64:65],
                        ones_f[:, 0:8].unsqueeze(2))

            # per-head scores/softmax/attn@v
            for ha in range(H):
                po = (ha % 2) * 64
                dl = ha // 2
                contrib = []
                for c in range(2):
                    sq_hi = c * 512 + 511
                    contrib.append([skt for skt in range(ST)
                                    if not (causal and skt * 128 > sq_hi)])
                rp = [ps_r.tile([128, 512], f32, tag="r", name=f"rp{_c}")
                      for _c in range(2)]
                for skt in range(ST):
                    cset = [c for c in range(2) if skt in contrib[c]]
                    if not cset:
                        continue
                    ats = {}
                    for c in cset:
                        cs = slice(c * 512, (c + 1) * 512)
                        sc = ps_a.tile([128, 512], f32, tag="a", name="sc")
                        mm(sc,
                           kT[po:po + 64, dl, skt * 128:(skt + 1) * 128],
                           qT[po:po + 64, dl, cs], True, True)
                        at = att.tile([128, 512], bf16, tag="at", name="at")
                        ats[c] = at
                        nc.scalar.activation(at, sc, AF.Exp, scale=ISQ)
                        if causal and skt * 128 + 127 > c * 512:
                            mi = (skt * 128 - c * 512) // 128
                            nc.vector.tensor_mul(at, at, masks[mi])
                    for c in cset:
                        mm(rp[c][0:65, :], vg[:, skt, ha, 0:65],
                           ats[c], skt == contrib[c][0], skt == contrib[c][-1])
                for c in range(2):
                    cs = slice(c * 512, (c + 1) * 512)
                    nc.vector.reciprocal(rec, rp[c][64:65, :])
                    bcr = ps_r.tile([128, 512], f32, tag="r")
                    mm(bcr[0:64, :], ones_row[:, 0:64], rec, True, True)
                    bcr_sb = sm.tile([64, 512], bf16, tag="bcr_sb", name="bcr_sb")
                    nc.vector.tensor_copy(bcr_sb, bcr[0:64, :])
                    nc.vector.tensor_mul(
                        repT[(ha % 2) * 64:(ha % 2) * 64 + 64, ha // 2, cs],
                        rp[c][0:64, :], bcr_sb)

            wot = load_w_full(wo_d, 0, 1024)
            proj_T(wot, bo, repT, dst_T, residual=residual_T)

        # ================= block body =================
        def block_body2(_i=None):
            # P0/P1: decoder stripes + LN1, transpose -> xT (A)
            for st in range(ST):
                stf = stri.tile([128, S], f32, tag="stripe_f")
                nc.sync.dma_start(stf, dec_d[st * 128:(st + 1) * 128, :])
                stats = sm.tile([128, 2, 6], f32, tag="bnst")
                xr = stf.rearrange("p (g d) -> p g d", g=2)
                for g2 in range(2):
                    nc.vector.bn_stats(stats[:, g2, :], xr[:, g2, :])
                mv = sm.tile([128, 2], f32, tag="bnmv")
                nc.vector.bn_aggr(mv, stats)
                std = sm.tile([128, 1], f32, tag="bnstd")
                nc.scalar.activation(std, mv[:, 1:2], AF.Sqrt, bias=eps128)
                rstd = sm.tile([128, 1], f32, tag="bnrstd")
                nc.vector.reciprocal(rstd, std)
                stb = stri.tile([128, S], bf16, tag="stripe_bf")
                nc.vector.tensor_scalar(stb, stf, mv[:, 0:1], rstd,
                                        ALU.subtract, ALU.mult)
                for j in range(DT):
                    tp = ps_r.tile([128, 512], f32, tag="r")
                    tpb = tp.bitcast(bf16)
                    nc.tensor.transpose(tpb[:, 0:128],
                                        stb[:, j * 128:(j + 1) * 128], ident_b)
                    nc.vector.tensor_scalar(
                        A[:, j, st * 128:(st + 1) * 128], tpb[:, 0:128],
                        bias_t['ln1_g'][:, j:j + 1], bias_t['ln1_b'][:, j:j + 1],
                        ALU.mult, ALU.add)

            # P3: self-attention (causal)
            attention(A, A, 'sa_', True, Bb, A, C, Hb, R)

            # P4: LN2
            ln_partition(Bb, bias_t['ln2_g'], bias_t['ln2_b'])

            # P4.5: encoder -> encT (A)
            for st in range(ST):
                stf = stri.tile([128, S], f32, tag="stripe_f")
                nc.sync.dma_start(stf.bitcast(f32r),
                                  enc_d[st * 128:(st + 1) * 128, :].bitcast(f32r))
                for j in range(DT):
                    tp = ps_r.tile([128, 512], f32, tag="r")
                    nc.tensor.transpose(tp[:, 0:128].bitcast(f32r),
                                        stf[:, j * 128:(j + 1) * 128].bitcast(f32r),
                                        ident_r)
                    nc.vector.tensor_copy(A[:, j, st * 128:(st + 1) * 128],
                                          tp[:, 0:128])

            # P5: cross-attention
            attention(Bb, A, 'ca_', False, C, Bb, C, Hb, R)

            # P6: LN3
            ln_partition(C, bias_t['ln3_g'], bias_t['ln3_b'])

            # P7: FFN, two FF halves; h in R(0:8)+Hb(8:16); accumulate into Bb
            for hf in range(2):
                for cc in range(2):
                    w1t = load_w_full(w1_d, (hf * 2 + cc) * 1024, 1024)
                    for mi8 in range(8):
                        ft = hf * 16 + cc * 8 + mi8
                        mi = cc * 8 + mi8
                        pss = [ps_a.tile([128, 512], f32, tag="a",
                                         name=f"f1{_c}") for _c in range(2)]
                        for k in range(DT):
                            for c in range(2):
                                cs = slice(c * 512, (c + 1) * 512)
                                mm(pss[c], w1t[:, k, mi8 * 128:(mi8 + 1) * 128],
                                   C[:, k, cs], k == 0, k == DT - 1)
                        hb = [R, Hb][mi // 8]
                        for c in range(2):
                            cs = slice(c * 512, (c + 1) * 512)
                            nc.scalar.activation(hb[:, mi % 8, cs], pss[c],
                                                 AF.Gelu,
                                                 bias=bias_t['ffn_b1'][:, ft:ft + 1],
                                                 scale=1.0)
                w2a = wf.tile([128, DT, 1024], bf16, tag="wf")
                nc.sync.dma_start(
                    w2a, w2_d[hf * 2048:hf * 2048 + 1024, :]
                    .rearrange("(k p) q -> p k q", p=128))
                w2b = wf.tile([128, DT, 1024], bf16, tag="wf")
                nc.sync.dma_start(
                    w2b, w2_d[hf * 2048 + 1024:hf * 2048 + 2048, :]
                    .rearrange("(k p) q -> p k q", p=128))
                for m in range(DT):
                    pss = [ps_a.tile([128, 512], f32, tag="a", name=f"f2{_c}")
                           for _c in range(2)]
                    for k2 in range(16):
                        w2t = w2a if k2 < 8 else w2b
                        hsrc = [R, Hb][k2 // 8]
                        for c in range(2):
                            cs = slice(c * 512, (c + 1) * 512)
                            mm(pss[c], w2t[:, k2 % 8, m * 128:(m + 1) * 128],
                               hsrc[:, k2 % 8, cs], k2 == 0, k2 == 15)
                    for c in range(2):
                        cs = slice(c * 512, (c + 1) * 512)
                        if hf == 0:
                            nc.vector.scalar_tensor_tensor(
                                Bb[:, m, cs], pss[c], 1.0, C[:, m, cs],
                                ALU.mult, ALU.add)
                        else:
                            nc.vector.scalar_tensor_tensor(
                                Bb[:, m, cs], pss[c], bias_t['ffn_b2'][:, m:m + 1],
                                Bb[:, m, cs], ALU.add, ALU.add)

            # P8: transpose Bb (outT, bf16) -> natural f32 stripes -> DRAM
            for j in range(ST):
                ost = stri.tile([128, S], f32, tag="stripe_f")
                for i in range(DT):
                    tp = ps_r.tile([128, 512], f32, tag="r")
                    tpb = tp.bitcast(bf16)
                    nc.tensor.transpose(tpb[:, 0:128],
                                        Bb[:, i, j * 128:(j + 1) * 128], ident_b)
                    nc.vector.tensor_copy(ost[:, i * 128:(i + 1) * 128],
                                          tpb[:, 0:128])
                nc.sync.dma_start(out_d[j * 128:(j + 1) * 128, :], ost)

        if iters == 1:
            block_body2()
        else:
            with tc.For_i(0, iters, 1):
                block_body2()


_CACHE = {}


def _get_nc(iters=1):
    if iters not in _CACHE:
        _CACHE[iters] = _build(iters)
    return _CACHE[iters]


def _in_maps(inputs):
    shared = {}
    for n in W_NAMES + ['ffn_w1', 'ffn_w2']:
        shared[n] = np.ascontiguousarray(
            np.asarray(inputs[n], dtype=np.float32).astype(ml_dtypes.bfloat16))
    for n in B_NAMES + LN_NAMES + ['ffn_b1', 'ffn_b2']:
        shared[n] = np.ascontiguousarray(np.asarray(inputs[n], dtype=np.float32))
    for n in ('sa_bv', 'ca_bv'):
        shared[n] = shared[n].reshape(1, D)
    dec = np.asarray(inputs['decoder'], dtype=np.float32)
    enc = np.asarray(inputs['encoder'], dtype=np.float32)
    maps = []
    for b in range(B):
        m = dict(shared)
        m['decoder'] = np.ascontiguousarray(dec[b])
        m['encoder'] = np.ascontiguousarray(enc[b])
        maps.append(m)
    return maps


def kernel(**inputs):
    nc = _get_nc(1)
    res = run_bass_kernel_spmd(nc, _in_maps(inputs), core_ids=list(range(B)))
    return np.stack([res.results[b]['out'] for b in range(B)], axis=0)
